# revision 19
# baseline (speedup 1.0000x reference)
"""Trainium2 Bass kernel for nn_AsymmetricLossCustomMS.

Reference math per sample b (x, y, y_neg: [B, C]; group_mask: [L, C]):
  xs     = sigmoid(x)
  thres  = max(16th-largest of xs, 0.3)
  gmax_l = max over classes in group l of xs        (L groups)
  gt_l   = any positive y in group l; gt_neg_l likewise for y_neg
  caseB  = sum_l rank_loss picked by gt_l           (if any gt_l)
  caseA  = mix of union-max and neg-score rank losses (otherwise)
  loss   = mean over b

Strategy: pure data parallel over the batch (256 rows/core on 8 cores).
sigmoid is monotonic, so the 16th-largest and the group maxima are taken on
raw x and sigmoided afterwards (tiny [128, L] tensors).

Layout trick: the host permutes x columns so the whitelist classes come
first, each group padded with -1e30 to a fixed W-wide segment.  The group
maxima are then in-place slices of the same x row-tile (no second copy of
the whitelist values over HBM), and a column permutation doesn't change the
row top-k.

16th-largest per row: pairwise tensor_tensor-max halvings (DVE runs those
at 2 elem/cycle for bf16, vs 1 for MAX8) shrink the 9728-wide row to 152
candidates, then MAX8 -> MATCH_REPLACE8 -> MAX8 gives the 16th-largest of
the folded array exactly.  Folding to 152 slots loses a top-16 member only
when two of them collide in one slot (E[collisions] ~ 0.8/row, and losing
one just promotes the 17th-largest -- error ~1e-3 in sigmoid space).
x streams in four chunks per row-tile (small ones first and last so the
pipeline starts early and the post-stream tail is short), each folded to
608 candidates on arrival and merged into a running accumulator.

Engine split: DVE owns the x folds, top-16, reduces, and the loss
arithmetic; the otherwise-idle Pool engine folds the whitelist group
segments and computes the boolean helpers; the scalar engine does the
sigmoids.  All DMAs go through the sync engine's hardware queue (gpsimd's
software DGE queue is ~15x slower).

y/y_neg: only whitelist columns matter; the host bit-packs them
(np.packbits, padded to 9 words) per group segment, and one DVE max-reduce
over uint32 words + not_equal recovers the per-group any-positive flags
for both row-tiles at once.
"""

import numpy as np

B, C, L = 2048, 9605, 8
N_CORES = 8
ROWS = B // N_CORES  # 256 rows per core
P = 128              # SBUF partitions per row-tile
TILES = ROWS // P    # 2 row-tiles per core
NEG = -1e30
ALPHA1 = 0.05  # margin
ALPHA3 = 5.0   # logistic sharpness
ALPHA_OTHER = 0.3
BIAS = ALPHA3 * ALPHA1

LAST_RESULT = None  # BassKernelResults of the most recent run (for test harness)

_graph_cache = {}


def _build(W, C_PAD):
    import concourse.bacc as bacc
    import concourse.tile as tile
    from concourse import mybir
    from concourse.alu_op_type import AluOpType as Op

    BF16 = mybir.dt.bfloat16
    F32 = mybir.dt.float32
    U32 = mybir.dt.uint32
    SIG = mybir.ActivationFunctionType.Sigmoid
    X = mybir.AxisListType.X

    S = C_PAD // 16      # fold-tree leaf width
    WQ = W // 32 + (1 if W % 32 else 0)  # y bit-words per segment
    NWL = L * W          # whitelist block width
    assert NWL == 4 * S  # whitelist block == chunk 0, group-aligned folds
    CHUNKS = [4, 2, 8, 2]  # x DMA chunks per tile, in S units
    BOUNDS = [0]
    for k in CHUNKS:
        BOUNDS.append(BOUNDS[-1] + k * S)

    nc = bacc.Bacc("TRN2", target_bir_lowering=False, debug=False, num_devices=N_CORES)
    x_d = nc.dram_tensor("x", [ROWS, C_PAD], BF16, kind="ExternalInput")
    zy_d = nc.dram_tensor("zy", [ROWS, 2 * L * WQ], U32, kind="ExternalInput")
    out_d = nc.dram_tensor("loss", [P, TILES], F32, kind="ExternalOutput")

    with tile.TileContext(nc) as tc:
        with tc.tile_pool(name="consts", bufs=1) as consts, \
             tc.tile_pool(name="xbuf", bufs=1) as xbuf, \
             tc.tile_pool(name="scr", bufs=1) as scr, \
             tc.tile_pool(name="sm", bufs=1) as sm:
            xt = [xbuf.tile([P, C_PAD], BF16, name=f"xt{t}") for t in range(TILES)]
            bias_c = consts.tile([P, 1], F32)
            nc.gpsimd.memset(bias_c, BIAS)

            # --- input DMAs on hardware queues, in consumption order.
            # the scalar engine's preamble finishes ~2us before sync's
            # first issue slot, so it launches the first two chunks; y bits
            # slot in just before the final x chunk.
            zy = sm.tile([P, TILES, 2 * L * WQ], U32)
            for t in range(TILES):
                for c in range(len(CHUNKS)):
                    if t == TILES - 1 and c == len(CHUNKS) - 1:
                        nc.sync.dma_start(
                            out=zy,
                            in_=zy_d.ap().rearrange("(t p) f -> p t f", t=TILES),
                        )
                    eng = nc.scalar if (t == 0 and c < 2) else nc.sync
                    eng.dma_start(
                        out=xt[t][:, BOUNDS[c]:BOUNDS[c + 1]],
                        in_=x_d.ap()[t * P:(t + 1) * P, BOUNDS[c]:BOUNDS[c + 1]],
                    )

            # --- shared small tensors (both tiles side by side in free dim)
            gm2 = sm.tile([P, TILES, L], F32)     # raw group maxima
            gsig2 = sm.tile([P, TILES, L], F32)   # sigmoid(group max)
            yr2 = sm.tile([P, TILES, 2 * L], U32)
            gt2 = sm.tile([P, TILES, 2 * L], F32)
            sgn2 = sm.tile([P, TILES, L], F32)
            negp2 = sm.tile([P, TILES, L], F32)
            un2 = sm.tile([P, TILES, 2], F32)
            dm2 = sm.tile([P, TILES, L], F32)
            dA2 = sm.tile([P, TILES, 2], F32)
            thr = [sm.tile([P, 1], F32, name=f"thr{t}") for t in range(TILES)]

            for t in range(TILES):
                xx = xt[t]
                # fold each chunk to S candidates by repeated halving, merge
                # into a running acc, then 608 -> 304 -> 152 for the top-16
                tmp = scr.tile([P, 4 * S], BF16, name=f"tmp_{t}")
                acc = scr.tile([P, S], BF16, name=f"acc_{t}")
                f2 = scr.tile([P, S // 2], BF16, name=f"f2_{t}")
                f4 = scr.tile([P, S // 4], BF16, name=f"f4_{t}")
                for c, k in enumerate(CHUNKS):
                    o = BOUNDS[c]
                    if c == 0:
                        # chunk 0 is the whitelist block: fold group-locally
                        # (4:1 into acc) so the group-max path reuses the
                        # main tree's first two levels.  Pool's ALU has no
                        # max, so all of this stays on DVE.
                        v = xx[:, 0:NWL].rearrange("p (g w) -> p g w", w=W)
                        z1 = scr.tile([P, L, W // 2], BF16, name=f"z1_{t}")
                        acc3 = acc.rearrange("p (g w) -> p g w", w=W // 4)
                        nc.vector.tensor_tensor(
                            out=z1, in0=v[:, :, 0:W // 2],
                            in1=v[:, :, W // 2:W], op=Op.max,
                        )
                        nc.vector.tensor_tensor(
                            out=acc3, in0=z1[:, :, 0:W // 4],
                            in1=z1[:, :, W // 4:W // 2], op=Op.max,
                        )
                        # group maxima: keep folding within segments (reads
                        # acc before any merge overwrites it)
                        z3 = scr.tile([P, L, W // 8], BF16, name=f"z3_{t}")
                        z4 = scr.tile([P, L, W // 16], BF16, name=f"z4_{t}")
                        z5 = scr.tile([P, L, W // 32], BF16, name=f"z5_{t}")
                        nc.vector.tensor_tensor(
                            out=z3, in0=acc3[:, :, 0:W // 8],
                            in1=acc3[:, :, W // 8:W // 4], op=Op.max,
                        )
                        nc.vector.tensor_tensor(
                            out=z4, in0=z3[:, :, 0:W // 16],
                            in1=z3[:, :, W // 16:W // 8], op=Op.max,
                        )
                        nc.vector.tensor_tensor(
                            out=z5, in0=z4[:, :, 0:W // 32],
                            in1=z4[:, :, W // 32:W // 16], op=Op.max,
                        )
                        nc.vector.tensor_reduce(
                            out=gm2[:, t], in_=z5, op=Op.max, axis=X
                        )
                        nc.scalar.activation(out=gsig2[:, t], in_=gm2[:, t], func=SIG)
                        continue
                    w = k * S
                    src = xx[:, o:o + w]
                    while w > S:
                        w //= 2
                        nc.vector.tensor_tensor(
                            out=tmp[:, 0:w], in0=src[:, 0:w], in1=src[:, w:2 * w],
                            op=Op.max,
                        )
                        src = tmp[:, 0:w]
                    nc.vector.tensor_tensor(
                        out=acc, in0=acc, in1=src[:, 0:S], op=Op.max
                    )
                    if c == 1 and t == 0:
                        yv = zy.rearrange("p t (g w) -> p t g w", w=WQ)
                        nc.vector.tensor_reduce(
                            out=yr2, in_=yv, op=Op.max, axis=X
                        )
                        nc.vector.tensor_scalar(
                            out=gt2, in0=yr2, scalar1=0, scalar2=None,
                            op0=Op.not_equal,
                        )
                        nc.gpsimd.tensor_scalar(
                            out=sgn2, in0=gt2[:, :, 0:L], scalar1=-2.0,
                            scalar2=1.0, op0=Op.mult, op1=Op.add,
                        )
                nc.vector.tensor_tensor(
                    out=f2, in0=acc[:, 0:S // 2], in1=acc[:, S // 2:S], op=Op.max
                )
                nc.vector.tensor_tensor(
                    out=f4, in0=f2[:, 0:S // 4], in1=f2[:, S // 4:S // 2], op=Op.max
                )
                # exact 16th-largest of the folded row
                g8 = sm.tile([P, 8], BF16, name=f"g8_{t}")
                nc.vector.max(out=g8, in_=f4)
                nc.vector.match_replace(
                    out=f4, in_to_replace=g8, in_values=f4, imm_value=NEG
                )
                n8 = sm.tile([P, 8], BF16, name=f"n8_{t}")
                nc.vector.max(out=n8, in_=f4)
                nc.scalar.activation(out=thr[t], in_=n8[:, 7:8], func=SIG)
                nc.vector.tensor_scalar_max(thr[t], thr[t], ALPHA_OTHER)

                # dm = (gsig - thres) * (1 - 2*gt);  dA = [umax, negscore] - thres
                nc.vector.scalar_tensor_tensor(
                    out=dm2[:, t], in0=gsig2[:, t], scalar=thr[t],
                    in1=sgn2[:, t], op0=Op.subtract, op1=Op.mult,
                )

            # --- combined tail over both tiles ---
            # union max and neg score
            nc.vector.reduce_max(out=un2[:, :, 0:1], in_=gsig2, axis=X)
            nc.gpsimd.tensor_tensor(
                out=negp2, in0=gt2[:, :, L:2 * L], in1=gsig2, op=Op.mult
            )
            nc.vector.reduce_max(out=un2[:, :, 1:2], in_=negp2, axis=X)
            for t in range(TILES):
                nc.vector.tensor_scalar(
                    out=dA2[:, t], in0=un2[:, t], scalar1=thr[t], scalar2=None,
                    op0=Op.subtract,
                )
            # caseB terms: sigmoid(5*dm + 0.25) * (1 + (dm > -0.05)), sum over l
            sB2 = sm.tile([P, TILES, L], F32)
            nc.scalar.activation(
                out=sB2, in_=dm2, func=SIG, scale=ALPHA3, bias=bias_c[:]
            )
            pB2 = sm.tile([P, TILES, L], F32)
            nc.gpsimd.tensor_scalar(
                out=pB2, in0=dm2, scalar1=-ALPHA1, scalar2=1.0,
                op0=Op.is_gt, op1=Op.add,
            )
            fB2 = sm.tile([P, TILES, L], F32)
            nc.vector.tensor_mul(fB2, sB2, pB2)
            caseB2 = sm.tile([P, TILES], F32)
            nc.vector.reduce_sum(out=caseB2, in_=fB2, axis=X)

            sA2 = sm.tile([P, TILES, 2], F32)
            nc.scalar.activation(
                out=sA2, in_=dA2, func=SIG, scale=ALPHA3, bias=bias_c[:]
            )
            pA2 = sm.tile([P, TILES, 2], F32)
            nc.gpsimd.tensor_scalar(
                out=pA2, in0=dA2, scalar1=-ALPHA1, scalar2=1.0,
                op0=Op.is_gt, op1=Op.add,
            )
            fA2 = sm.tile([P, TILES, 2], F32)
            nc.vector.tensor_mul(fA2, sA2, pA2)
            caseAr2 = sm.tile([P, TILES], F32)
            nc.vector.reduce_sum(out=caseAr2, in_=fA2, axis=X)
            caseA2 = sm.tile([P, TILES], F32)
            nc.gpsimd.tensor_scalar(
                out=caseA2, in0=caseAr2, scalar1=0.5, scalar2=None, op0=Op.mult
            )

            # loss = caseA + has_gt * (caseB - caseA);  caseA = 0.5 * caseAr
            hg2 = sm.tile([P, TILES], F32)
            nc.vector.reduce_max(out=hg2, in_=gt2[:, :, 0:L], axis=X)
            dd2 = sm.tile([P, TILES], F32)
            nc.vector.scalar_tensor_tensor(
                out=dd2, in0=caseAr2, scalar=-0.5, in1=caseB2,
                op0=Op.mult, op1=Op.add,
            )
            nc.vector.tensor_mul(dd2, dd2, hg2)
            lossr2 = sm.tile([P, TILES], F32)
            nc.vector.tensor_add(lossr2, caseA2, dd2)
            nc.sync.dma_start(out=out_d.ap(), in_=lossr2)
    nc.compile()
    return nc


def _reset_device():
    """Best-effort recovery of a wedged axon-tunneled NeuronCore."""
    import ctypes
    import time

    try:
        import jax

        jax.devices()
        lib = ctypes.CDLL("/opt/axon/libaxon_pjrt.so")
        lib.axon_reset.restype = ctypes.c_int64
        lib.axon_reset()
        time.sleep(45)
    except Exception:
        pass


def kernel(x, y, y_neg, group_mask):
    global LAST_RESULT
    from concourse.bass_utils import run_bass_kernel_spmd
    import ml_dtypes

    x = np.asarray(x, dtype=np.float32)
    y = np.asarray(y, dtype=np.float32)
    y_neg = np.asarray(y_neg, dtype=np.float32)
    gm = np.asarray(group_mask).astype(bool)
    BF16 = ml_dtypes.bfloat16

    cols = [np.flatnonzero(gm[l]) for l in range(L)]
    wmax = max((len(c) for c in cols), default=1)
    wl = np.concatenate(cols) if cols else np.zeros(0, np.int64)
    other = np.flatnonzero(~gm.any(axis=0))
    # W sized so the whitelist block (8 segments) is exactly chunk 0 (4
    # leaves of the 16-leaf fold tree, C_PAD = 32W) and the rest fits too
    W = 32
    while W < wmax or 24 * W < len(other):
        W += 32
    WB = W // 8                      # packed bit-bytes per segment
    WQ = WB // 4                     # uint32 words per segment
    NWL = L * W
    C_PAD = 32 * W                   # 16 leaves of S = 2W
    assert NWL + len(other) <= C_PAD
    dst_wl = np.concatenate(
        [l * W + np.arange(len(cl)) for l, cl in enumerate(cols)]
    )

    # x with whitelist groups gathered to the front, NEG-padded segments
    xp = np.full((B, C_PAD), NEG, dtype=BF16)
    xb = x.astype(BF16)
    xp[:, dst_wl] = xb[:, wl]
    xp[:, NWL:NWL + len(other)] = xb[:, other]

    # bit-packed y / y_neg whitelist columns: [B, 2L, WQ] uint32 words
    ybits = np.zeros((B, 2 * NWL), dtype=bool)
    ybits[:, dst_wl] = y[:, wl] != 0
    ybits[:, NWL + dst_wl] = y_neg[:, wl] != 0
    packed = np.packbits(ybits.reshape(B, 2 * L, W), axis=-1)  # [B, 2L, WB]
    padded = np.zeros((B, 2 * L, 4 * WQ), dtype=np.uint8)
    padded[:, :, :WB] = packed
    zyb = padded.view(np.uint32).reshape(B, 2 * L * WQ)

    key = (W, C_PAD)
    if key not in _graph_cache:
        _graph_cache[key] = _build(W, C_PAD)
    nc = _graph_cache[key]

    in_maps = [
        {"x": xp[i * ROWS:(i + 1) * ROWS], "zy": zyb[i * ROWS:(i + 1) * ROWS]}
        for i in range(N_CORES)
    ]
    try:
        res = run_bass_kernel_spmd(nc, in_maps, core_ids=list(range(N_CORES)))
    except Exception:
        _reset_device()
        res = run_bass_kernel_spmd(nc, in_maps, core_ids=list(range(N_CORES)))
    LAST_RESULT = res

    loss = np.concatenate([res.results[i]["loss"].reshape(-1) for i in range(N_CORES)])
    return np.asarray(loss.mean(), dtype=np.float32)


# revision 20
# speedup vs baseline: 1.0318x; 1.0318x over previous
"""Trainium2 Bass kernel for nn_AsymmetricLossCustomMS.

Reference math per sample b (x, y, y_neg: [B, C]; group_mask: [L, C]):
  xs     = sigmoid(x)
  thres  = max(16th-largest of xs, 0.3)
  gmax_l = max over classes in group l of xs        (L groups)
  gt_l   = any positive y in group l; gt_neg_l likewise for y_neg
  caseB  = sum_l rank_loss picked by gt_l           (if any gt_l)
  caseA  = mix of union-max and neg-score rank losses (otherwise)
  loss   = mean over b

Strategy: pure data parallel over the batch (256 rows/core on 8 cores).
sigmoid is monotonic, so the 16th-largest and the group maxima are taken on
raw x and sigmoided afterwards (tiny [128, L] tensors).

Layout trick: the host permutes x columns so the whitelist classes come
first, each group padded with -1e30 to a fixed W-wide segment.  The group
maxima are then in-place slices of the same x row-tile (no second copy of
the whitelist values over HBM), and a column permutation doesn't change the
row top-k.

16th-largest per row: pairwise tensor_tensor-max halvings (DVE runs those
at 2 elem/cycle for bf16, vs 1 for MAX8) shrink the 9728-wide row to 152
candidates, then MAX8 -> MATCH_REPLACE8 -> MAX8 gives the 16th-largest of
the folded array exactly.  Folding to 152 slots loses a top-16 member only
when two of them collide in one slot (E[collisions] ~ 0.8/row, and losing
one just promotes the 17th-largest -- error ~1e-3 in sigmoid space).
x streams in four chunks per row-tile (small ones first and last so the
pipeline starts early and the post-stream tail is short), each folded to
608 candidates on arrival and merged into a running accumulator.

Engine split: DVE owns the x folds, top-16, reduces, and the loss
arithmetic; the otherwise-idle Pool engine folds the whitelist group
segments and computes the boolean helpers; the scalar engine does the
sigmoids.  All DMAs go through the sync engine's hardware queue (gpsimd's
software DGE queue is ~15x slower).

y/y_neg: only whitelist columns matter; the host bit-packs them
(np.packbits, padded to 9 words) per group segment, and one DVE max-reduce
over uint32 words + not_equal recovers the per-group any-positive flags
for both row-tiles at once.
"""

import numpy as np

B, C, L = 2048, 9605, 8
N_CORES = 8
ROWS = B // N_CORES  # 256 rows per core
P = 128              # SBUF partitions per row-tile
TILES = ROWS // P    # 2 row-tiles per core
NEG = -1e30
ALPHA1 = 0.05  # margin
ALPHA3 = 5.0   # logistic sharpness
ALPHA_OTHER = 0.3
BIAS = ALPHA3 * ALPHA1

LAST_RESULT = None  # BassKernelResults of the most recent run (for test harness)

_graph_cache = {}


def _build(W, C_PAD):
    import concourse.bacc as bacc
    import concourse.tile as tile
    from concourse import mybir
    from concourse.alu_op_type import AluOpType as Op

    BF16 = mybir.dt.bfloat16
    F32 = mybir.dt.float32
    U32 = mybir.dt.uint32
    SIG = mybir.ActivationFunctionType.Sigmoid
    X = mybir.AxisListType.X

    S = C_PAD // 16      # fold-tree leaf width
    WQ = W // 32 + (1 if W % 32 else 0)  # y bit-words per segment
    NWL = L * W          # whitelist block width
    assert NWL == 4 * S  # whitelist block == chunk 1, group-aligned folds
    WLOFF = 2 * S        # whitelist block offset (a small chunk leads)
    CHUNKS = [2, 4, 8, 2]  # x DMA chunks per tile, in S units
    BOUNDS = [0]
    for k in CHUNKS:
        BOUNDS.append(BOUNDS[-1] + k * S)

    nc = bacc.Bacc("TRN2", target_bir_lowering=False, debug=False, num_devices=N_CORES)
    x_d = nc.dram_tensor("x", [ROWS, C_PAD], BF16, kind="ExternalInput")
    zy_d = nc.dram_tensor("zy", [ROWS, 2 * L * WQ], U32, kind="ExternalInput")
    out_d = nc.dram_tensor("loss", [P, TILES], F32, kind="ExternalOutput")

    with tile.TileContext(nc) as tc:
        with tc.tile_pool(name="consts", bufs=1) as consts, \
             tc.tile_pool(name="xbuf", bufs=1) as xbuf, \
             tc.tile_pool(name="scr", bufs=1) as scr, \
             tc.tile_pool(name="sm", bufs=1) as sm:
            xt = [xbuf.tile([P, C_PAD], BF16, name=f"xt{t}") for t in range(TILES)]
            bias_c = consts.tile([P, 1], F32)
            nc.gpsimd.memset(bias_c, BIAS)

            # --- input DMAs, all on the sync engine's single hardware
            # queue: it serves transfers at full fabric bandwidth in issue
            # order (a second queue halves the rate of both); y bits slot
            # in just before the final x chunk.
            zy = sm.tile([P, TILES, 2 * L * WQ], U32)
            for t in range(TILES):
                for c in range(len(CHUNKS)):
                    if t == TILES - 1 and c == len(CHUNKS) - 1:
                        nc.sync.dma_start(
                            out=zy,
                            in_=zy_d.ap().rearrange("(t p) f -> p t f", t=TILES),
                        )
                    nc.sync.dma_start(
                        out=xt[t][:, BOUNDS[c]:BOUNDS[c + 1]],
                        in_=x_d.ap()[t * P:(t + 1) * P, BOUNDS[c]:BOUNDS[c + 1]],
                    )

            # --- shared small tensors (both tiles side by side in free dim)
            gm2 = sm.tile([P, TILES, L], F32)     # raw group maxima
            gsig2 = sm.tile([P, TILES, L], F32)   # sigmoid(group max)
            yr2 = sm.tile([P, TILES, 2 * L], U32)
            gt2 = sm.tile([P, TILES, 2 * L], F32)
            sgn2 = sm.tile([P, TILES, L], F32)
            negp2 = sm.tile([P, TILES, L], F32)
            un2 = sm.tile([P, TILES, 2], F32)
            dm2 = sm.tile([P, TILES, L], F32)
            dA2 = sm.tile([P, TILES, 2], F32)
            thr = [sm.tile([P, 1], F32, name=f"thr{t}") for t in range(TILES)]

            for t in range(TILES):
                xx = xt[t]
                # fold each chunk to S candidates by repeated halving, merge
                # into a running acc, then 608 -> 304 -> 152 for the top-16
                tmp = scr.tile([P, 4 * S], BF16, name=f"tmp_{t}")
                acc = scr.tile([P, S], BF16, name=f"acc_{t}")
                f2 = scr.tile([P, S // 2], BF16, name=f"f2_{t}")
                f4 = scr.tile([P, S // 4], BF16, name=f"f4_{t}")
                wlf = scr.tile([P, S], BF16, name=f"wlf_{t}")
                for c, k in enumerate(CHUNKS):
                    o = BOUNDS[c]
                    if c == 1:
                        # chunk 1 is the whitelist block: fold group-locally
                        # (4:1) so the group-max path reuses the main
                        # tree's first two levels.  Pool's ALU has no max,
                        # so all of this stays on DVE.
                        v = xx[:, WLOFF:WLOFF + NWL].rearrange(
                            "p (g w) -> p g w", w=W
                        )
                        z1 = scr.tile([P, L, W // 2], BF16, name=f"z1_{t}")
                        wlf3 = wlf.rearrange("p (g w) -> p g w", w=W // 4)
                        nc.vector.tensor_tensor(
                            out=z1, in0=v[:, :, 0:W // 2],
                            in1=v[:, :, W // 2:W], op=Op.max,
                        )
                        nc.vector.tensor_tensor(
                            out=wlf3, in0=z1[:, :, 0:W // 4],
                            in1=z1[:, :, W // 4:W // 2], op=Op.max,
                        )
                        nc.vector.tensor_tensor(
                            out=acc, in0=acc, in1=wlf, op=Op.max
                        )
                        # group maxima: keep folding within segments
                        z3 = scr.tile([P, L, W // 8], BF16, name=f"z3_{t}")
                        z4 = scr.tile([P, L, W // 16], BF16, name=f"z4_{t}")
                        z5 = scr.tile([P, L, W // 32], BF16, name=f"z5_{t}")
                        nc.vector.tensor_tensor(
                            out=z3, in0=wlf3[:, :, 0:W // 8],
                            in1=wlf3[:, :, W // 8:W // 4], op=Op.max,
                        )
                        nc.vector.tensor_tensor(
                            out=z4, in0=z3[:, :, 0:W // 16],
                            in1=z3[:, :, W // 16:W // 8], op=Op.max,
                        )
                        nc.vector.tensor_tensor(
                            out=z5, in0=z4[:, :, 0:W // 32],
                            in1=z4[:, :, W // 32:W // 16], op=Op.max,
                        )
                        nc.vector.tensor_reduce(
                            out=gm2[:, t], in_=z5, op=Op.max, axis=X
                        )
                        nc.scalar.activation(out=gsig2[:, t], in_=gm2[:, t], func=SIG)
                        if t == 0:
                            yv = zy.rearrange("p t (g w) -> p t g w", w=WQ)
                            nc.vector.tensor_reduce(
                                out=yr2, in_=yv, op=Op.max, axis=X
                            )
                            nc.vector.tensor_scalar(
                                out=gt2, in0=yr2, scalar1=0, scalar2=None,
                                op0=Op.not_equal,
                            )
                            nc.gpsimd.tensor_scalar(
                                out=sgn2, in0=gt2[:, :, 0:L], scalar1=-2.0,
                                scalar2=1.0, op0=Op.mult, op1=Op.add,
                            )
                        continue
                    w = k * S
                    src = xx[:, o:o + w]
                    while w > S:
                        w //= 2
                        dst = (acc if c == 0 else tmp)[:, 0:w] if w == S \
                            else tmp[:, 0:w]
                        nc.vector.tensor_tensor(
                            out=dst, in0=src[:, 0:w], in1=src[:, w:2 * w],
                            op=Op.max,
                        )
                        src = dst
                    if c > 0:
                        nc.vector.tensor_tensor(
                            out=acc, in0=acc, in1=src[:, 0:S], op=Op.max
                        )
                nc.vector.tensor_tensor(
                    out=f2, in0=acc[:, 0:S // 2], in1=acc[:, S // 2:S], op=Op.max
                )
                nc.vector.tensor_tensor(
                    out=f4, in0=f2[:, 0:S // 4], in1=f2[:, S // 4:S // 2], op=Op.max
                )
                # exact 16th-largest of the folded row
                g8 = sm.tile([P, 8], BF16, name=f"g8_{t}")
                nc.vector.max(out=g8, in_=f4)
                nc.vector.match_replace(
                    out=f4, in_to_replace=g8, in_values=f4, imm_value=NEG
                )
                n8 = sm.tile([P, 8], BF16, name=f"n8_{t}")
                nc.vector.max(out=n8, in_=f4)
                nc.scalar.activation(out=thr[t], in_=n8[:, 7:8], func=SIG)
                nc.vector.tensor_scalar_max(thr[t], thr[t], ALPHA_OTHER)

                # dm = (gsig - thres) * (1 - 2*gt);  dA = [umax, negscore] - thres
                nc.vector.scalar_tensor_tensor(
                    out=dm2[:, t], in0=gsig2[:, t], scalar=thr[t],
                    in1=sgn2[:, t], op0=Op.subtract, op1=Op.mult,
                )

            # --- combined tail over both tiles ---
            # union max and neg score
            nc.vector.reduce_max(out=un2[:, :, 0:1], in_=gsig2, axis=X)
            nc.gpsimd.tensor_tensor(
                out=negp2, in0=gt2[:, :, L:2 * L], in1=gsig2, op=Op.mult
            )
            nc.vector.reduce_max(out=un2[:, :, 1:2], in_=negp2, axis=X)
            for t in range(TILES):
                nc.vector.tensor_scalar(
                    out=dA2[:, t], in0=un2[:, t], scalar1=thr[t], scalar2=None,
                    op0=Op.subtract,
                )
            # caseB terms: sigmoid(5*dm + 0.25) * (1 + (dm > -0.05)), sum over l
            sB2 = sm.tile([P, TILES, L], F32)
            nc.scalar.activation(
                out=sB2, in_=dm2, func=SIG, scale=ALPHA3, bias=bias_c[:]
            )
            pB2 = sm.tile([P, TILES, L], F32)
            nc.gpsimd.tensor_scalar(
                out=pB2, in0=dm2, scalar1=-ALPHA1, scalar2=1.0,
                op0=Op.is_gt, op1=Op.add,
            )
            fB2 = sm.tile([P, TILES, L], F32)
            nc.vector.tensor_mul(fB2, sB2, pB2)
            caseB2 = sm.tile([P, TILES], F32)
            nc.vector.reduce_sum(out=caseB2, in_=fB2, axis=X)

            sA2 = sm.tile([P, TILES, 2], F32)
            nc.scalar.activation(
                out=sA2, in_=dA2, func=SIG, scale=ALPHA3, bias=bias_c[:]
            )
            pA2 = sm.tile([P, TILES, 2], F32)
            nc.gpsimd.tensor_scalar(
                out=pA2, in0=dA2, scalar1=-ALPHA1, scalar2=1.0,
                op0=Op.is_gt, op1=Op.add,
            )
            fA2 = sm.tile([P, TILES, 2], F32)
            nc.vector.tensor_mul(fA2, sA2, pA2)
            caseAr2 = sm.tile([P, TILES], F32)
            nc.vector.reduce_sum(out=caseAr2, in_=fA2, axis=X)
            caseA2 = sm.tile([P, TILES], F32)
            nc.gpsimd.tensor_scalar(
                out=caseA2, in0=caseAr2, scalar1=0.5, scalar2=None, op0=Op.mult
            )

            # loss = caseA + has_gt * (caseB - caseA);  caseA = 0.5 * caseAr
            hg2 = sm.tile([P, TILES], F32)
            nc.vector.reduce_max(out=hg2, in_=gt2[:, :, 0:L], axis=X)
            dd2 = sm.tile([P, TILES], F32)
            nc.vector.scalar_tensor_tensor(
                out=dd2, in0=caseAr2, scalar=-0.5, in1=caseB2,
                op0=Op.mult, op1=Op.add,
            )
            nc.vector.tensor_mul(dd2, dd2, hg2)
            lossr2 = sm.tile([P, TILES], F32)
            nc.vector.tensor_add(lossr2, caseA2, dd2)
            nc.sync.dma_start(out=out_d.ap(), in_=lossr2)
    nc.compile()
    return nc


def _reset_device():
    """Best-effort recovery of a wedged axon-tunneled NeuronCore."""
    import ctypes
    import time

    try:
        import jax

        jax.devices()
        lib = ctypes.CDLL("/opt/axon/libaxon_pjrt.so")
        lib.axon_reset.restype = ctypes.c_int64
        lib.axon_reset()
        time.sleep(45)
    except Exception:
        pass


def kernel(x, y, y_neg, group_mask):
    global LAST_RESULT
    from concourse.bass_utils import run_bass_kernel_spmd
    import ml_dtypes

    x = np.asarray(x, dtype=np.float32)
    y = np.asarray(y, dtype=np.float32)
    y_neg = np.asarray(y_neg, dtype=np.float32)
    gm = np.asarray(group_mask).astype(bool)
    BF16 = ml_dtypes.bfloat16

    cols = [np.flatnonzero(gm[l]) for l in range(L)]
    wmax = max((len(c) for c in cols), default=1)
    wl = np.concatenate(cols) if cols else np.zeros(0, np.int64)
    other = np.flatnonzero(~gm.any(axis=0))
    # W sized so the whitelist block (8 segments) is exactly chunk 0 (4
    # leaves of the 16-leaf fold tree, C_PAD = 32W) and the rest fits too
    W = 32
    while W < wmax or 24 * W < len(other):
        W += 32
    WB = W // 8                      # packed bit-bytes per segment
    WQ = WB // 4                     # uint32 words per segment
    NWL = L * W
    C_PAD = 32 * W                   # 16 leaves of S = 2W
    assert NWL + len(other) <= C_PAD
    dst_wl = np.concatenate(
        [l * W + np.arange(len(cl)) for l, cl in enumerate(cols)]
    )

    # x layout: [other 2S | whitelist block | rest of other | pad], the
    # whitelist groups NEG-padded to W-wide segments
    S = 2 * W
    xp = np.full((B, C_PAD), NEG, dtype=BF16)
    xb = x.astype(BF16)
    xp[:, 0:2 * S] = xb[:, other[0:2 * S]]
    xp[:, 2 * S + dst_wl] = xb[:, wl]
    xp[:, 2 * S + NWL:2 * S + NWL + len(other) - 2 * S] = xb[:, other[2 * S:]]

    # bit-packed y / y_neg whitelist columns: [B, 2L, WQ] uint32 words
    ybits = np.zeros((B, 2 * NWL), dtype=bool)
    ybits[:, dst_wl] = y[:, wl] != 0
    ybits[:, NWL + dst_wl] = y_neg[:, wl] != 0
    packed = np.packbits(ybits.reshape(B, 2 * L, W), axis=-1)  # [B, 2L, WB]
    padded = np.zeros((B, 2 * L, 4 * WQ), dtype=np.uint8)
    padded[:, :, :WB] = packed
    zyb = padded.view(np.uint32).reshape(B, 2 * L * WQ)

    key = (W, C_PAD)
    if key not in _graph_cache:
        _graph_cache[key] = _build(W, C_PAD)
    nc = _graph_cache[key]

    in_maps = [
        {"x": xp[i * ROWS:(i + 1) * ROWS], "zy": zyb[i * ROWS:(i + 1) * ROWS]}
        for i in range(N_CORES)
    ]
    try:
        res = run_bass_kernel_spmd(nc, in_maps, core_ids=list(range(N_CORES)))
    except Exception:
        _reset_device()
        res = run_bass_kernel_spmd(nc, in_maps, core_ids=list(range(N_CORES)))
    LAST_RESULT = res

    loss = np.concatenate([res.results[i]["loss"].reshape(-1) for i in range(N_CORES)])
    return np.asarray(loss.mean(), dtype=np.float32)


# revision 21
# speedup vs baseline: 1.0645x; 1.0316x over previous
"""Trainium2 Bass kernel for nn_AsymmetricLossCustomMS.

Reference math per sample b (x, y, y_neg: [B, C]; group_mask: [L, C]):
  xs     = sigmoid(x)
  thres  = max(16th-largest of xs, 0.3)
  gmax_l = max over classes in group l of xs        (L groups)
  gt_l   = any positive y in group l; gt_neg_l likewise for y_neg
  caseB  = sum_l rank_loss picked by gt_l           (if any gt_l)
  caseA  = mix of union-max and neg-score rank losses (otherwise)
  loss   = mean over b

Strategy: pure data parallel over the batch (256 rows/core on 8 cores).
sigmoid is monotonic, so the 16th-largest and the group maxima are taken on
raw x and sigmoided afterwards (tiny [128, L] tensors).

Layout trick: the host permutes x columns so the whitelist classes come
first, each group padded with -1e30 to a fixed W-wide segment.  The group
maxima are then in-place slices of the same x row-tile (no second copy of
the whitelist values over HBM), and a column permutation doesn't change the
row top-k.

16th-largest per row: pairwise tensor_tensor-max halvings (DVE runs those
at 2 elem/cycle for bf16, vs 1 for MAX8) shrink the 9728-wide row to 152
candidates, then MAX8 -> MATCH_REPLACE8 -> MAX8 gives the 16th-largest of
the folded array exactly.  Folding to 152 slots loses a top-16 member only
when two of them collide in one slot (E[collisions] ~ 0.8/row, and losing
one just promotes the 17th-largest -- error ~1e-3 in sigmoid space).
x streams in five chunks per row-tile (small ones first and last so the
pipeline starts early and the post-stream tail is short), each folded to
608 candidates on arrival and merged into a running accumulator.

Engine split: DVE owns all the max work (Pool's ALU has no max) plus the
loss arithmetic; Pool computes the mult/compare helpers; the scalar engine
does the sigmoids.  All DMAs ride the sync engine's single hardware-DGE
queue -- it serves transfers at full fabric bandwidth in issue order
(gpsimd's software queue dribbles ~15x slower, and a second hardware queue
halves the rate of both).

y/y_neg: only whitelist columns matter; the host bit-packs them
(np.packbits, padded to whole words) per group segment, and one DVE
max-reduce over uint32 words + not_equal recovers the per-group
any-positive flags for both row-tiles at once.
"""

import numpy as np

B, C, L = 2048, 9605, 8
N_CORES = 8
ROWS = B // N_CORES  # 256 rows per core
P = 128              # SBUF partitions per row-tile
TILES = ROWS // P    # 2 row-tiles per core
NEG = -1e30
ALPHA1 = 0.05  # margin
ALPHA3 = 5.0   # logistic sharpness
ALPHA_OTHER = 0.3
BIAS = ALPHA3 * ALPHA1

LAST_RESULT = None  # BassKernelResults of the most recent run (for test harness)

_graph_cache = {}


def _build(W, C_PAD):
    import concourse.bacc as bacc
    import concourse.tile as tile
    from concourse import mybir
    from concourse.alu_op_type import AluOpType as Op

    BF16 = mybir.dt.bfloat16
    F32 = mybir.dt.float32
    U32 = mybir.dt.uint32
    SIG = mybir.ActivationFunctionType.Sigmoid
    X = mybir.AxisListType.X

    S = C_PAD // 16      # fold-tree leaf width
    WQ = W // 32 + (1 if W % 32 else 0)  # y bit-words per segment
    NWL = L * W          # whitelist block width
    assert NWL <= 4 * S  # whitelist block inside the first two chunks
    CHUNKS = [2, 2, 2, 8, 2]  # x DMA chunks per tile, in S units
    BOUNDS = [0]
    for k in CHUNKS:
        BOUNDS.append(BOUNDS[-1] + k * S)

    nc = bacc.Bacc("TRN2", target_bir_lowering=False, debug=False, num_devices=N_CORES)
    x_d = nc.dram_tensor("x", [ROWS, C_PAD], BF16, kind="ExternalInput")
    zy_d = nc.dram_tensor("zy", [ROWS, 2 * L * WQ], U32, kind="ExternalInput")
    out_d = nc.dram_tensor("loss", [P, TILES], F32, kind="ExternalOutput")

    with tile.TileContext(nc) as tc:
        with tc.tile_pool(name="consts", bufs=1) as consts, \
             tc.tile_pool(name="xbuf", bufs=1) as xbuf, \
             tc.tile_pool(name="scr", bufs=1) as scr, \
             tc.tile_pool(name="sm", bufs=1) as sm:
            xt = [xbuf.tile([P, C_PAD], BF16, name=f"xt{t}") for t in range(TILES)]
            bias_c = consts.tile([P, 1], F32)
            nc.gpsimd.memset(bias_c, BIAS)

            # --- input DMAs, all on the sync engine's hardware queue, in
            # consumption order; y bits slot in just before the final x
            # chunk (their consumers need a little lead time).
            zy = sm.tile([P, TILES, 2 * L * WQ], U32)
            for t in range(TILES):
                for c in range(len(CHUNKS)):
                    if t == TILES - 1 and c == len(CHUNKS) - 1:
                        nc.sync.dma_start(
                            out=zy,
                            in_=zy_d.ap().rearrange("(t p) f -> p t f", t=TILES),
                        )
                    nc.sync.dma_start(
                        out=xt[t][:, BOUNDS[c]:BOUNDS[c + 1]],
                        in_=x_d.ap()[t * P:(t + 1) * P, BOUNDS[c]:BOUNDS[c + 1]],
                    )

            # --- shared small tensors (both tiles side by side in free dim)
            gm2 = sm.tile([P, TILES, L], F32)     # raw group maxima
            gsig2 = sm.tile([P, TILES, L], F32)   # sigmoid(group max)
            yr2 = sm.tile([P, TILES, 2 * L], U32)
            gt2 = sm.tile([P, TILES, 2 * L], F32)
            sgn2 = sm.tile([P, TILES, L], F32)
            negp2 = sm.tile([P, TILES, L], F32)
            un2 = sm.tile([P, TILES, 2], F32)
            dm2 = sm.tile([P, TILES, L], F32)
            dA2 = sm.tile([P, TILES, 2], F32)
            thr = [sm.tile([P, 1], F32, name=f"thr{t}") for t in range(TILES)]

            for t in range(TILES):
                xx = xt[t]
                # fold each chunk to S candidates by repeated halving, merge
                # into a running acc, then 608 -> 304 -> 152 for the top-16
                tmp = scr.tile([P, 4 * S], BF16, name=f"tmp_{t}")
                acc = scr.tile([P, S], BF16, name=f"acc_{t}")
                f2 = scr.tile([P, S // 2], BF16, name=f"f2_{t}")
                f4 = scr.tile([P, S // 4], BF16, name=f"f4_{t}")
                for c, k in enumerate(CHUNKS):
                    o = BOUNDS[c]
                    w = k * S
                    src = xx[:, o:o + w]
                    while w > S:
                        w //= 2
                        dst = (acc if c == 0 else tmp)[:, 0:w] if w == S \
                            else tmp[:, 0:w]
                        nc.vector.tensor_tensor(
                            out=dst, in0=src[:, 0:w], in1=src[:, w:2 * w],
                            op=Op.max,
                        )
                        src = dst
                    if c > 0:
                        nc.vector.tensor_tensor(
                            out=acc, in0=acc, in1=src[:, 0:S], op=Op.max
                        )
                    if c == 1:
                        # group-max folds over the whitelist segments (they
                        # live inside chunks 0-1); Pool's ALU has no max, so
                        # these stay on DVE in the chunk-wait slack.
                        v = xx[:, 0:NWL].rearrange("p (g w) -> p g w", w=W)
                        z1 = scr.tile([P, L, W // 2], BF16, name=f"z1_{t}")
                        z2 = scr.tile([P, L, W // 4], BF16, name=f"z2_{t}")
                        z3 = scr.tile([P, L, W // 8], BF16, name=f"z3_{t}")
                        nc.vector.tensor_tensor(
                            out=z1, in0=v[:, :, 0:W // 2],
                            in1=v[:, :, W // 2:W], op=Op.max,
                        )
                        nc.vector.tensor_tensor(
                            out=z2, in0=z1[:, :, 0:W // 4],
                            in1=z1[:, :, W // 4:W // 2], op=Op.max,
                        )
                        nc.vector.tensor_tensor(
                            out=z3, in0=z2[:, :, 0:W // 8],
                            in1=z2[:, :, W // 8:W // 4], op=Op.max,
                        )
                        nc.vector.tensor_reduce(
                            out=gm2[:, t], in_=z3, op=Op.max, axis=X
                        )
                        nc.scalar.activation(out=gsig2[:, t], in_=gm2[:, t], func=SIG)
                        if t == 0:
                            yv = zy.rearrange("p t (g w) -> p t g w", w=WQ)
                            nc.vector.tensor_reduce(
                                out=yr2, in_=yv, op=Op.max, axis=X
                            )
                            nc.vector.tensor_scalar(
                                out=gt2, in0=yr2, scalar1=0, scalar2=None,
                                op0=Op.not_equal,
                            )
                            nc.gpsimd.tensor_scalar(
                                out=sgn2, in0=gt2[:, :, 0:L], scalar1=-2.0,
                                scalar2=1.0, op0=Op.mult, op1=Op.add,
                            )
                nc.vector.tensor_tensor(
                    out=f2, in0=acc[:, 0:S // 2], in1=acc[:, S // 2:S], op=Op.max
                )
                nc.vector.tensor_tensor(
                    out=f4, in0=f2[:, 0:S // 4], in1=f2[:, S // 4:S // 2], op=Op.max
                )
                # exact 16th-largest of the folded row
                g8 = sm.tile([P, 8], BF16, name=f"g8_{t}")
                nc.vector.max(out=g8, in_=f4)
                nc.vector.match_replace(
                    out=f4, in_to_replace=g8, in_values=f4, imm_value=NEG
                )
                n8 = sm.tile([P, 8], BF16, name=f"n8_{t}")
                nc.vector.max(out=n8, in_=f4)
                nc.scalar.activation(out=thr[t], in_=n8[:, 7:8], func=SIG)
                nc.vector.tensor_scalar_max(thr[t], thr[t], ALPHA_OTHER)

                # dm = (gsig - thres) * (1 - 2*gt)
                nc.vector.scalar_tensor_tensor(
                    out=dm2[:, t], in0=gsig2[:, t], scalar=thr[t],
                    in1=sgn2[:, t], op0=Op.subtract, op1=Op.mult,
                )

            # --- combined tail over both tiles ---
            # union max and neg score
            nc.vector.reduce_max(out=un2[:, :, 0:1], in_=gsig2, axis=X)
            nc.gpsimd.tensor_tensor(
                out=negp2, in0=gt2[:, :, L:2 * L], in1=gsig2, op=Op.mult
            )
            nc.vector.reduce_max(out=un2[:, :, 1:2], in_=negp2, axis=X)
            for t in range(TILES):
                nc.vector.tensor_scalar(
                    out=dA2[:, t], in0=un2[:, t], scalar1=thr[t], scalar2=None,
                    op0=Op.subtract,
                )
            # caseB terms: sigmoid(5*dm + 0.25) * (1 + (dm > -0.05)), sum over l
            sB2 = sm.tile([P, TILES, L], F32)
            nc.scalar.activation(
                out=sB2, in_=dm2, func=SIG, scale=ALPHA3, bias=bias_c[:]
            )
            pB2 = sm.tile([P, TILES, L], F32)
            nc.gpsimd.tensor_scalar(
                out=pB2, in0=dm2, scalar1=-ALPHA1, scalar2=1.0,
                op0=Op.is_gt, op1=Op.add,
            )
            fB2 = sm.tile([P, TILES, L], F32)
            nc.vector.tensor_mul(fB2, sB2, pB2)
            caseB2 = sm.tile([P, TILES], F32)
            nc.vector.reduce_sum(out=caseB2, in_=fB2, axis=X)

            sA2 = sm.tile([P, TILES, 2], F32)
            nc.scalar.activation(
                out=sA2, in_=dA2, func=SIG, scale=ALPHA3, bias=bias_c[:]
            )
            pA2 = sm.tile([P, TILES, 2], F32)
            nc.gpsimd.tensor_scalar(
                out=pA2, in0=dA2, scalar1=-ALPHA1, scalar2=1.0,
                op0=Op.is_gt, op1=Op.add,
            )
            fA2 = sm.tile([P, TILES, 2], F32)
            nc.vector.tensor_mul(fA2, sA2, pA2)
            caseAr2 = sm.tile([P, TILES], F32)
            nc.vector.reduce_sum(out=caseAr2, in_=fA2, axis=X)
            caseA2 = sm.tile([P, TILES], F32)
            nc.gpsimd.tensor_scalar(
                out=caseA2, in0=caseAr2, scalar1=0.5, scalar2=None, op0=Op.mult
            )

            # loss = caseA + has_gt * (caseB - caseA);  caseA = 0.5 * caseAr
            hg2 = sm.tile([P, TILES], F32)
            nc.vector.reduce_max(out=hg2, in_=gt2[:, :, 0:L], axis=X)
            dd2 = sm.tile([P, TILES], F32)
            nc.vector.scalar_tensor_tensor(
                out=dd2, in0=caseAr2, scalar=-0.5, in1=caseB2,
                op0=Op.mult, op1=Op.add,
            )
            nc.vector.tensor_mul(dd2, dd2, hg2)
            lossr2 = sm.tile([P, TILES], F32)
            nc.vector.tensor_add(lossr2, caseA2, dd2)
            nc.sync.dma_start(out=out_d.ap(), in_=lossr2)
    nc.compile()
    return nc


def _reset_device():
    """Best-effort recovery of a wedged axon-tunneled NeuronCore."""
    import ctypes
    import time

    try:
        import jax

        jax.devices()
        lib = ctypes.CDLL("/opt/axon/libaxon_pjrt.so")
        lib.axon_reset.restype = ctypes.c_int64
        lib.axon_reset()
        time.sleep(45)
    except Exception:
        pass


def kernel(x, y, y_neg, group_mask):
    global LAST_RESULT
    from concourse.bass_utils import run_bass_kernel_spmd
    import ml_dtypes

    x = np.asarray(x, dtype=np.float32)
    y = np.asarray(y, dtype=np.float32)
    y_neg = np.asarray(y_neg, dtype=np.float32)
    gm = np.asarray(group_mask).astype(bool)
    BF16 = ml_dtypes.bfloat16

    cols = [np.flatnonzero(gm[l]) for l in range(L)]
    wmax = max((len(c) for c in cols), default=1)
    W = ((max(wmax, 1) + 7) // 8) * 8
    WB = W // 8                      # packed bit-bytes per segment
    WQ = WB // 4 + (1 if WB % 4 else 0)  # uint32 words per segment
    NWL = L * W
    wl = np.concatenate(cols) if cols else np.zeros(0, np.int64)
    other = np.flatnonzero(~gm.any(axis=0))
    n_cols = NWL + len(other)
    C_PAD = ((n_cols + 15) // 16) * 16
    dst_wl = np.concatenate(
        [l * W + np.arange(len(cl)) for l, cl in enumerate(cols)]
    )

    # x with whitelist groups gathered to the front, NEG-padded segments
    xp = np.full((B, C_PAD), NEG, dtype=BF16)
    xb = x.astype(BF16)
    xp[:, dst_wl] = xb[:, wl]
    xp[:, NWL:NWL + len(other)] = xb[:, other]

    # bit-packed y / y_neg whitelist columns: [B, 2L, WQ] uint32 words
    ybits = np.zeros((B, 2 * NWL), dtype=bool)
    ybits[:, dst_wl] = y[:, wl] != 0
    ybits[:, NWL + dst_wl] = y_neg[:, wl] != 0
    packed = np.packbits(ybits.reshape(B, 2 * L, W), axis=-1)  # [B, 2L, WB]
    padded = np.zeros((B, 2 * L, 4 * WQ), dtype=np.uint8)
    padded[:, :, :WB] = packed
    zyb = padded.view(np.uint32).reshape(B, 2 * L * WQ)

    key = (W, C_PAD)
    if key not in _graph_cache:
        _graph_cache[key] = _build(W, C_PAD)
    nc = _graph_cache[key]

    in_maps = [
        {"x": xp[i * ROWS:(i + 1) * ROWS], "zy": zyb[i * ROWS:(i + 1) * ROWS]}
        for i in range(N_CORES)
    ]
    try:
        res = run_bass_kernel_spmd(nc, in_maps, core_ids=list(range(N_CORES)))
    except Exception:
        _reset_device()
        res = run_bass_kernel_spmd(nc, in_maps, core_ids=list(range(N_CORES)))
    LAST_RESULT = res

    loss = np.concatenate([res.results[i]["loss"].reshape(-1) for i in range(N_CORES)])
    return np.asarray(loss.mean(), dtype=np.float32)
